# revision 14
# baseline (speedup 1.0000x reference)
"""Trainium2 Bass kernel for nn_LuenbergerLDS (B=32, T=2048, N=512, M=512).

Math: the reference is a diagonal complex linear recurrence followed by a
projection; since d == 1 the whole module is a causal LTI SIMO filter
    y[t, b, m] = sum_{j>=0} H[j, m] * x[t - j, b] + Do[m]
with impulse response H computed on host in float64 from (lam, Winv, C, D).

Key structure exploited here (vs the previous all-direct-FIR version):
the FIR tail H[LH:LH+WT] is numerically low-rank (singular values fall
below 1e-3 of ||y|| by index ~48), so it is factored H_tail ~= U @ V
(rank R) via SVD on host.  The device then computes, per output chunk of
128 timesteps, a SINGLE 128x512 fp16 matmul whose stationary operand
stacks [x head lags (LH=80) ; tail coefficients z (R=48)] and whose
moving operand stacks [H_head ; V].  z itself comes from a cheap "basis
conv" stage: 4 matmuls per 512-timestep superchunk contracting lag tiles
of U against diagonal (Toeplitz) slices of x.

The z coefficients are written (fp32->fp16 copy) into partitions 80..127
of the SAME per-batch diagonal x-buffer that serves the head lags in
partitions 0..79, so the output matmul's stationary operand is a plain
contiguous slice.  The diagonal buffer xsh[p, v] = xpad[v + p] makes
both the conv moving slices and the head/stacked stationary slices
simple strided views.

dtype: fp16 everywhere on the PE (10 mantissa bits; measured end-to-end
error 6.7e-4 of max|y| vs 2e-2 tolerance).  PSUM accumulates fp32.
Output is written fp16 and upcast on host (adds Do there too).

Per core (4 batches): 16 conv matmuls + 16 output matmuls per batch =
128 matmuls-512 total (vs 452 in the previous version).  Output DMAs are
staged 4 output tiles per dma_start to amortize DGE cost; evacuation
copies rotate across DVE/Pool/Activation engines.
"""

import os
import sys

sys.path.insert(0, "/opt/trn_rl_repo")

import numpy as np

# problem dims (hardcoded per harness contract)
B, T, N, M = 32, 2048, 512, 512
NCORES = 8
BLOC = B // NCORES          # batches per core
LH = 64                     # direct head lags [0, LH); partition-quadrant aligned
R = 128 - LH                # tail rank (stacked into the same 128 contraction)
NTILE = 4                   # conv lag tiles of 128 -> tail window
WT = NTILE * 128            # tail lags [LH, LH+WT)
RPAD = LH + WT - 1          # 591: left zero pad of x
NV = T + 512                # diag buffer v-range [0, NV)
XROWS = NV + 127            # xpad rows so the diag load never reads OOB
NCH = T // 128              # output chunks per batch
NSC = T // 512              # superchunks per batch

MODE = os.environ.get("K_MODE", "f16")
NLAG = NTILE  # kept for test.py cache-key compatibility


def build_program(mode=MODE):
    import concourse.tile as tile
    from concourse import bacc, mybir

    f16 = mybir.dt.float16
    f32 = mybir.dt.float32

    nc = bacc.Bacc("TRN2", target_bir_lowering=False, debug=False)
    xpad_t = nc.dram_tensor("xpad", [BLOC, XROWS], f16, kind="ExternalInput")
    hcomb_t = nc.dram_tensor("hcomb", [128, M], f16, kind="ExternalInput")
    ucomb_t = nc.dram_tensor("ucomb", [128, NTILE * R], f16, kind="ExternalInput")
    y_t = nc.dram_tensor("y", [BLOC, T, M], f16, kind="ExternalOutput")

    VCH = 512                   # v-granularity of xsh loads (1KB descriptors)
    with tile.TileContext(nc) as tc:
        with (
            tc.tile_pool(name="xsh", bufs=1) as xsh_pool,
            tc.tile_pool(name="w", bufs=1) as wpool,
            tc.tile_pool(name="psum", bufs=1, space="PSUM") as psum_pool,
            tc.tile_pool(name="stage", bufs=1) as stage_pool,
        ):
            # ---- load plan (critical-first, 3 queues round-robin) ----
            xsh = []
            for b in range(BLOC):
                t_ = xsh_pool.tile([128, NV], f16, tag=f"xshb{b}", name=f"xsh{b}")
                xsh.append(t_)
            ucomb_sb = wpool.tile([128, NTILE * R], f16, tag="ucomb", name="ucomb_sb")
            hcomb_sb = wpool.tile([128, M], f16, tag="hcomb", name="hcomb_sb")

            engines = [nc.sync, nc.scalar, nc.gpsimd]
            ei = 0

            def dma(eng, out_ap, in_ap):
                eng.dma_start(out=out_ap, in_=in_ap)

            def load_xchunk(b, v0, eng):
                nvc = min(VCH, NV - v0)
                in_ap = xpad_t.ap().copy()
                from bass_rust import VecI64Pair
                in_ap.ap = VecI64Pair([[1, 128], [1, nvc]])
                in_ap.offset = b * XROWS + v0
                dma(eng, xsh[b][:, v0 : v0 + nvc], in_ap)

            # ucomb first (needed by the very first conv matmul); first x
            # chunks on sync+gpsimd (scalar's queue is blocked early by its
            # ACT_TABLE_LOAD).
            dma(nc.sync, ucomb_sb[:], ucomb_t.ap())
            dma(nc.scalar, hcomb_sb[:], hcomb_t.ap())
            nvch = (NV + VCH - 1) // VCH
            le01 = [nc.sync, nc.gpsimd]
            le23 = [nc.sync, nc.gpsimd, nc.scalar]
            for v in range(nvch):
                for b in (0, 1):
                    load_xchunk(b, v * VCH, le01[ei % 2])
                    ei += 1
            ei = 0
            for v in range(nvch):
                for b in (2, 3):
                    load_xchunk(b, v * VCH, le23[ei % 3])
                    ei += 1

            # ---- compute ----
            # NOTE: GpSimd (Pool) cannot access PSUM, so evacuation copies
            # rotate across DVE + Activation only; y-write DMA issue goes to
            # SP + Pool to keep those two engines free for copies.
            evac_engines = [nc.vector, nc.scalar]
            ydma_engines = [nc.sync, nc.gpsimd]
            ci = 0
            yi = 0
            def zcopy(b, s, zt):
                nonlocal ci
                eng = evac_engines[ci % 2]
                ci += 1
                w0 = 512 + 512 * s
                if eng is nc.scalar:
                    eng.copy(xsh[b][LH:128, w0 : w0 + 512], zt[LH:128, :])
                else:
                    eng.tensor_copy(xsh[b][LH:128, w0 : w0 + 512], zt[LH:128, :])

            prevzt = {}

            def conv_group(b, s):
                # write z^T into PSUM partitions 64..127 directly
                # (tile_position col=64) so the copy to SBUF partitions
                # 64..127 never crosses partitions.
                zt = psum_pool.tile([128, M], f32, tag="zt", bufs=3, name="zt")
                for i, l in enumerate(range(NTILE - 1, -1, -1)):
                    v = 384 + 512 * s - 128 * l
                    nc.tensor.matmul(
                        zt[LH:128, :],
                        lhsT=ucomb_sb[:, l * R : (l + 1) * R],
                        rhs=xsh[b][:, v : v + 512],
                        start=(i == 0),
                        stop=(i == NTILE - 1),
                    )
                # CRITICAL ORDER: zcopy(b, s) overwrites diag-buffer cells
                # (partitions 64..127, window [512+512s, 1024+512s)) that
                # conv(b, s+1) still reads as x data, so zcopy(b, s-1) is
                # emitted only now (after this group's matmuls), and
                # zcopy(b, NSC-1) right after its own group.
                if s > 0:
                    zcopy(b, s - 1, prevzt[b])
                prevzt[b] = zt
                if s == NSC - 1:
                    zcopy(b, s, zt)

            from bass_rust import VecI64Pair

            def out_group(b, chunks, ydma_eng=None):
                osb = stage_pool.tile(
                    [128, 4 * M], f16, tag="osb", bufs=3, name="osb"
                )
                nonlocal ci, yi
                for k, c in enumerate(chunks):
                    w = 512 + 128 * c
                    ot = psum_pool.tile([128, M], f32, tag="ot", bufs=5, name="ot")
                    nc.tensor.matmul(
                        ot[:],
                        lhsT=xsh[b][:, w : w + 128],
                        rhs=hcomb_sb[:],
                        start=True,
                        stop=True,
                    )
                    eng = evac_engines[ci % 2]
                    ci += 1
                    if eng is nc.scalar:
                        eng.copy(osb[:, k * M : (k + 1) * M], ot[:])
                    else:
                        eng.tensor_copy(osb[:, k * M : (k + 1) * M], ot[:])
                dst = y_t.ap().copy()
                dst.ap = VecI64Pair([[M, 128], [128 * M, len(chunks)], [1, M]])
                dst.offset = b * T * M + chunks[0] * 128 * M
                eng = ydma_eng or ydma_engines[yi % 2]
                yi += 1
                eng.dma_start(out=dst, in_=osb[:, : len(chunks) * M])

            # Interleave: conv(b0) alone, then conv(b_{k+1}) groups alternate
            # with out(b_k) groups so the 2 evac engines (the real
            # throughput limit of the out phase) stay evenly loaded.
            for s in range(NSC):
                conv_group(0, s)
            for k in range(3):
                for s in range(NSC):
                    conv_group(k + 1, s)
                    out_group(k, [4 * s + j for j in range(4)])
            # tail batch: shrink the final staged writes so the kernel
            # drain (last evac + last y DMA) is short; route the last two
            # DMAs to the otherwise-idle scalar HW queue and sync.
            b = 3
            out_group(b, [0, 1, 2, 3])
            out_group(b, [4, 5, 6, 7])
            out_group(b, [8, 9, 10, 11])
            out_group(b, [12, 13])
            out_group(b, [14], ydma_eng=nc.scalar)
            out_group(b, [15], ydma_eng=nc.sync)

    nc.compile()
    return nc


def host_weights(lnl_re, lnl_im, W_r, W_i, C, D, Do, mode=MODE):
    """Impulse response head + SVD-factored tail, float64 math."""
    lnl = lnl_re.astype(np.float64) + 1j * lnl_im.astype(np.float64)
    W = W_r.astype(np.float64) + 1j * W_i.astype(np.float64)
    Winv = np.linalg.inv(W)
    A_re = np.ascontiguousarray(Winv.real.T) @ C.astype(np.float64)
    A_im = np.ascontiguousarray(Winv.imag.T) @ C.astype(np.float64)
    j = np.arange(LH + WT, dtype=np.float64)
    P = np.exp(np.outer(j, lnl))
    H = P.real @ A_re - P.imag @ A_im                 # (LH+WT, M)
    H[0] += D[0].astype(np.float64)

    Hh = H[:LH]
    U, S, Vt = np.linalg.svd(H[LH:], full_matrices=False)
    sq = np.sqrt(S[:R])
    Uf = U[:, :R] * sq                                # (WT, R)
    Vf = sq[:, None] * Vt[:R]                         # (R, M)

    hcomb = np.concatenate([Hh[::-1], Vf], axis=0).astype(np.float16)
    ucomb = (
        Uf.reshape(NTILE, 128, R)[:, ::-1, :]
        .transpose(1, 0, 2)
        .reshape(128, NTILE * R)
        .astype(np.float16)
    )
    return {
        "hcomb": np.ascontiguousarray(hcomb),
        "ucomb": np.ascontiguousarray(ucomb),
    }


def make_in_maps(x, weights):
    x16 = x[:, :, 0].astype(np.float16)               # (B, T)
    in_maps = []
    for c in range(NCORES):
        xpad = np.zeros((BLOC, XROWS), np.float16)
        xpad[:, RPAD : RPAD + T] = x16[c * BLOC : (c + 1) * BLOC]
        im = dict(weights)
        im["xpad"] = xpad
        in_maps.append(im)
    return in_maps


_prog_cache = {}


def kernel(x, lnl_re, lnl_im, W_r, W_i, C, D, Do):
    from concourse.bass_utils import run_bass_kernel_spmd

    x = np.asarray(x)
    lnl_re, lnl_im = np.asarray(lnl_re), np.asarray(lnl_im)
    W_r, W_i = np.asarray(W_r), np.asarray(W_i)
    C, D, Do = np.asarray(C), np.asarray(D), np.asarray(Do)

    key = (NLAG, MODE)
    if key not in _prog_cache:
        _prog_cache[key] = build_program()
    nc = _prog_cache[key]

    weights = host_weights(lnl_re, lnl_im, W_r, W_i, C, D, Do)
    in_maps = make_in_maps(x, weights)
    res = run_bass_kernel_spmd(nc, in_maps, core_ids=list(range(NCORES)))
    y = np.concatenate([res.results[i]["y"] for i in range(NCORES)], axis=0)
    y = y.astype(np.float32) + Do.astype(np.float32)[None, None, :]
    return np.ascontiguousarray(y)


# revision 16
# speedup vs baseline: 1.0483x; 1.0483x over previous
"""Trainium2 Bass kernel for nn_LuenbergerLDS (B=32, T=2048, N=512, M=512).

Math: the reference is a diagonal complex linear recurrence followed by a
projection; since d == 1 the whole module is a causal LTI SIMO filter
    y[t, b, m] = sum_{j>=0} H[j, m] * x[t - j, b] + Do[m]
with impulse response H computed on host in float64 from (lam, Winv, C, D).

Key structure exploited here (vs the previous all-direct-FIR version):
the FIR tail H[LH:LH+WT] is numerically low-rank (singular values fall
below 1e-3 of ||y|| by index ~48), so it is factored H_tail ~= U @ V
(rank R) via SVD on host.  The device then computes, per output chunk of
128 timesteps, a SINGLE 128x512 fp16 matmul whose stationary operand
stacks [x head lags (LH=80) ; tail coefficients z (R=48)] and whose
moving operand stacks [H_head ; V].  z itself comes from a cheap "basis
conv" stage: 4 matmuls per 512-timestep superchunk contracting lag tiles
of U against diagonal (Toeplitz) slices of x.

The z coefficients are written (fp32->fp16 copy) into partitions 80..127
of the SAME per-batch diagonal x-buffer that serves the head lags in
partitions 0..79, so the output matmul's stationary operand is a plain
contiguous slice.  The diagonal buffer xsh[p, v] = xpad[v + p] makes
both the conv moving slices and the head/stacked stationary slices
simple strided views.

dtype: fp16 everywhere on the PE (10 mantissa bits; measured end-to-end
error 6.7e-4 of max|y| vs 2e-2 tolerance).  PSUM accumulates fp32.
Output is written fp16 and upcast on host (adds Do there too).

Per core (4 batches): 16 conv matmuls + 16 output matmuls per batch =
128 matmuls-512 total (vs 452 in the previous version).  Output DMAs are
staged 4 output tiles per dma_start to amortize DGE cost; evacuation
copies rotate across DVE/Pool/Activation engines.
"""

import os
import sys

sys.path.insert(0, "/opt/trn_rl_repo")

import numpy as np

# problem dims (hardcoded per harness contract)
B, T, N, M = 32, 2048, 512, 512
NCORES = 8
BLOC = B // NCORES          # batches per core
LH = 64                     # direct head lags [0, LH); partition-quadrant aligned
R = 128 - LH                # tail rank (stacked into the same 128 contraction)
NTILE = 4                   # conv lag tiles of 128 -> tail window
WT = NTILE * 128            # tail lags [LH, LH+WT)
RPAD = LH + WT - 1          # 591: left zero pad of x
NV = T + 512                # diag buffer v-range [0, NV)
XROWS = NV + 127            # xpad rows so the diag load never reads OOB
NCH = T // 128              # output chunks per batch
NSC = T // 512              # superchunks per batch

MODE = os.environ.get("K_MODE", "f16")
NLAG = NTILE  # kept for test.py cache-key compatibility


def build_program(mode=MODE):
    import concourse.tile as tile
    from concourse import bacc, mybir

    f16 = mybir.dt.float16
    f32 = mybir.dt.float32

    nc = bacc.Bacc("TRN2", target_bir_lowering=False, debug=False)
    xpad_t = nc.dram_tensor("xpad", [BLOC, XROWS], f16, kind="ExternalInput")
    hcomb_t = nc.dram_tensor("hcomb", [128, M], f16, kind="ExternalInput")
    ucomb_t = nc.dram_tensor("ucomb", [128, NTILE * R], f16, kind="ExternalInput")
    y_t = nc.dram_tensor("y", [BLOC, T, M], f16, kind="ExternalOutput")

    VCH = 512                   # v-granularity of xsh loads (1KB descriptors)
    with tile.TileContext(nc) as tc:
        with (
            tc.tile_pool(name="xsh", bufs=1) as xsh_pool,
            tc.tile_pool(name="w", bufs=1) as wpool,
            tc.tile_pool(name="psum", bufs=1, space="PSUM") as psum_pool,
            tc.tile_pool(name="stage", bufs=1) as stage_pool,
        ):
            # ---- load plan (critical-first, 3 queues round-robin) ----
            xsh = []
            for b in range(BLOC):
                t_ = xsh_pool.tile([128, NV], f16, tag=f"xshb{b}", name=f"xsh{b}")
                xsh.append(t_)
            ucomb_sb = wpool.tile([128, NTILE * R], f16, tag="ucomb", name="ucomb_sb")
            hcomb_sb = wpool.tile([128, M], f16, tag="hcomb", name="hcomb_sb")

            engines = [nc.sync, nc.scalar, nc.gpsimd]
            ei = 0

            def dma(eng, out_ap, in_ap):
                eng.dma_start(out=out_ap, in_=in_ap)

            def load_xchunk(b, v0, eng):
                nvc = min(VCH, NV - v0)
                in_ap = xpad_t.ap().copy()
                from bass_rust import VecI64Pair
                in_ap.ap = VecI64Pair([[1, 128], [1, nvc]])
                in_ap.offset = b * XROWS + v0
                dma(eng, xsh[b][:, v0 : v0 + nvc], in_ap)

            # ucomb first (needed by the very first conv matmul); first x
            # chunks on sync+gpsimd (scalar's queue is blocked early by its
            # ACT_TABLE_LOAD).
            dma(nc.sync, ucomb_sb[:], ucomb_t.ap())
            dma(nc.scalar, hcomb_sb[:], hcomb_t.ap())
            nvch = (NV + VCH - 1) // VCH
            le01 = [nc.sync, nc.gpsimd]
            le23 = [nc.sync, nc.gpsimd, nc.scalar]
            for v in range(nvch):
                for b in (0, 1):
                    load_xchunk(b, v * VCH, le01[ei % 2])
                    ei += 1
            ei = 0
            for v in range(nvch):
                for b in (2, 3):
                    load_xchunk(b, v * VCH, le23[ei % 3])
                    ei += 1

            # ---- compute ----
            # NOTE: GpSimd (Pool) cannot access PSUM, so evacuation copies
            # rotate across DVE + Activation only; y-write DMA issue goes to
            # SP + Pool to keep those two engines free for copies.
            evac_engines = [nc.vector, nc.scalar]
            ydma_engines = [nc.sync, nc.gpsimd]
            ci = 0
            yi = 0
            def zcopy(b, s, zt):
                nonlocal ci
                eng = evac_engines[ci % 2]
                ci += 1
                w0 = 512 + 512 * s
                if eng is nc.scalar:
                    eng.copy(xsh[b][LH:128, w0 : w0 + 512], zt[LH:128, :M])
                else:
                    eng.tensor_copy(xsh[b][LH:128, w0 : w0 + 512], zt[LH:128, :M])

            prevzt = {}

            def conv_group(b, s):
                # write z^T into PSUM partitions 64..127 directly
                # (tile_position col=64) so the copy to SBUF partitions
                # 64..127 never crosses partitions.  All PSUM goes through
                # one [128, 2M] (2-bank) tag so conv and out phases share
                # the 8 banks without overcommitting.
                zt = psum_pool.tile([128, 2 * M], f32, tag="pp", bufs=4, name="zt")
                for i, l in enumerate(range(NTILE - 1, -1, -1)):
                    v = 384 + 512 * s - 128 * l
                    nc.tensor.matmul(
                        zt[LH:128, :M],
                        lhsT=ucomb_sb[:, l * R : (l + 1) * R],
                        rhs=xsh[b][:, v : v + 512],
                        start=(i == 0),
                        stop=(i == NTILE - 1),
                    )
                # CRITICAL ORDER: zcopy(b, s) overwrites diag-buffer cells
                # (partitions 64..127, window [512+512s, 1024+512s)) that
                # conv(b, s+1) still reads as x data, so zcopy(b, s-1) is
                # emitted only now (after this group's matmuls), and
                # zcopy(b, NSC-1) right after its own group.
                if s > 0:
                    zcopy(b, s - 1, prevzt[b])
                prevzt[b] = zt
                if s == NSC - 1:
                    zcopy(b, s, zt)

            from bass_rust import VecI64Pair

            def out_pair(b, c0, osb, off, split=False):
                # two output-chunk matmuls into one 2-bank PSUM tile, then a
                # single fused (2-chunk-wide) evacuation copy -- halves the
                # per-op fixed cost on the two PSUM-capable engines.  With
                # split=True the two halves are copied by both engines in
                # parallel instead (shorter latency for the kernel tail).
                nonlocal ci
                pp = psum_pool.tile([128, 2 * M], f32, tag="pp", bufs=4, name="pp")
                for k in (0, 1):
                    w = 512 + 128 * (c0 + k)
                    nc.tensor.matmul(
                        pp[:, k * M : (k + 1) * M],
                        lhsT=xsh[b][:, w : w + 128],
                        rhs=hcomb_sb[:],
                        start=True,
                        stop=True,
                    )
                if split:
                    nc.vector.tensor_copy(osb[:, off : off + M], pp[:, :M])
                    nc.scalar.copy(osb[:, off + M : off + 2 * M], pp[:, M:])
                else:
                    eng = evac_engines[ci % 2]
                    ci += 1
                    if eng is nc.scalar:
                        eng.copy(osb[:, off : off + 2 * M], pp[:])
                    else:
                        eng.tensor_copy(osb[:, off : off + 2 * M], pp[:])

            def out_group(b, g, ydma_eng=None, tail=False):
                # one staged y write of 4 output chunks (2 fused pairs)
                nonlocal yi
                tag, bufs = ("osbt", 2) if tail else ("osb", 3)
                osb = stage_pool.tile([128, 4 * M], f16, tag=tag, bufs=bufs, name=tag)
                out_pair(b, 4 * g, osb, 0, split=tail)
                out_pair(b, 4 * g + 2, osb, 2 * M, split=tail)
                dst = y_t.ap().copy()
                dst.ap = VecI64Pair([[M, 128], [128 * M, 4], [1, M]])
                dst.offset = b * T * M + g * 512 * M
                eng = ydma_eng or ydma_engines[yi % 2]
                yi += 1
                eng.dma_start(out=dst, in_=osb[:])

            # Schedule: conv(pair1) -> [out(pair1) interleaved with
            # conv(pair2)] (that section is PE-bound: evac demand of the out
            # groups fits beside the conv groups' zcopies) -> out(pair2)
            # with a short-latency tail.
            for s in range(NSC):
                for b in (0, 1):
                    conv_group(b, s)
            og = [(b, g) for b in (0, 1) for g in range(NCH // 4)]
            cg = [(b, s) for s in range(NSC) for b in (2, 3)]
            for i in range(len(og)):
                out_group(*og[i])
                conv_group(*cg[i])
            for g in range(NCH // 4):
                out_group(2, g)
                if g < 3:
                    out_group(3, g)
            # kernel tail: parallel split evacs, dedicated staging buffers,
            # final DMAs on the two least-backlogged queues.
            out_group(3, 3, ydma_eng=nc.scalar, tail=True)

    nc.compile()
    return nc


def host_weights(lnl_re, lnl_im, W_r, W_i, C, D, Do, mode=MODE):
    """Impulse response head + SVD-factored tail, float64 math."""
    lnl = lnl_re.astype(np.float64) + 1j * lnl_im.astype(np.float64)
    W = W_r.astype(np.float64) + 1j * W_i.astype(np.float64)
    Winv = np.linalg.inv(W)
    A_re = np.ascontiguousarray(Winv.real.T) @ C.astype(np.float64)
    A_im = np.ascontiguousarray(Winv.imag.T) @ C.astype(np.float64)
    j = np.arange(LH + WT, dtype=np.float64)
    P = np.exp(np.outer(j, lnl))
    H = P.real @ A_re - P.imag @ A_im                 # (LH+WT, M)
    H[0] += D[0].astype(np.float64)

    Hh = H[:LH]
    U, S, Vt = np.linalg.svd(H[LH:], full_matrices=False)
    sq = np.sqrt(S[:R])
    Uf = U[:, :R] * sq                                # (WT, R)
    Vf = sq[:, None] * Vt[:R]                         # (R, M)

    hcomb = np.concatenate([Hh[::-1], Vf], axis=0).astype(np.float16)
    ucomb = (
        Uf.reshape(NTILE, 128, R)[:, ::-1, :]
        .transpose(1, 0, 2)
        .reshape(128, NTILE * R)
        .astype(np.float16)
    )
    return {
        "hcomb": np.ascontiguousarray(hcomb),
        "ucomb": np.ascontiguousarray(ucomb),
    }


def make_in_maps(x, weights):
    x16 = x[:, :, 0].astype(np.float16)               # (B, T)
    in_maps = []
    for c in range(NCORES):
        xpad = np.zeros((BLOC, XROWS), np.float16)
        xpad[:, RPAD : RPAD + T] = x16[c * BLOC : (c + 1) * BLOC]
        im = dict(weights)
        im["xpad"] = xpad
        in_maps.append(im)
    return in_maps


_prog_cache = {}


def kernel(x, lnl_re, lnl_im, W_r, W_i, C, D, Do):
    from concourse.bass_utils import run_bass_kernel_spmd

    x = np.asarray(x)
    lnl_re, lnl_im = np.asarray(lnl_re), np.asarray(lnl_im)
    W_r, W_i = np.asarray(W_r), np.asarray(W_i)
    C, D, Do = np.asarray(C), np.asarray(D), np.asarray(Do)

    key = (NLAG, MODE)
    if key not in _prog_cache:
        _prog_cache[key] = build_program()
    nc = _prog_cache[key]

    weights = host_weights(lnl_re, lnl_im, W_r, W_i, C, D, Do)
    in_maps = make_in_maps(x, weights)
    res = run_bass_kernel_spmd(nc, in_maps, core_ids=list(range(NCORES)))
    y = np.concatenate([res.results[i]["y"] for i in range(NCORES)], axis=0)
    y = y.astype(np.float32) + Do.astype(np.float32)[None, None, :]
    return np.ascontiguousarray(y)


# revision 23
# speedup vs baseline: 1.0683x; 1.0190x over previous
"""Trainium2 Bass kernel for nn_LuenbergerLDS (B=32, T=2048, N=512, M=512).

Math: the reference is a diagonal complex linear recurrence followed by a
projection; since d == 1 the whole module is a causal LTI SIMO filter
    y[t, b, m] = sum_{j>=0} H[j, m] * x[t - j, b] + Do[m]
with impulse response H computed on host in float64 from (lam, Winv, C, D).

Key structure exploited here (vs the previous all-direct-FIR version):
the FIR tail H[LH:LH+WT] is numerically low-rank (singular values fall
below 1e-3 of ||y|| by index ~48), so it is factored H_tail ~= U @ V
(rank R) via SVD on host.  The device then computes, per output chunk of
128 timesteps, a SINGLE 128x512 fp16 matmul whose stationary operand
stacks [x head lags (LH=80) ; tail coefficients z (R=48)] and whose
moving operand stacks [H_head ; V].  z itself comes from a cheap "basis
conv" stage: 4 matmuls per 512-timestep superchunk contracting lag tiles
of U against diagonal (Toeplitz) slices of x.

The z coefficients are written (fp32->fp16 copy) into partitions 80..127
of the SAME per-batch diagonal x-buffer that serves the head lags in
partitions 0..79, so the output matmul's stationary operand is a plain
contiguous slice.  The diagonal buffer xsh[p, v] = xpad[v + p] makes
both the conv moving slices and the head/stacked stationary slices
simple strided views.

dtype: fp16 everywhere on the PE (10 mantissa bits; measured end-to-end
error 6.7e-4 of max|y| vs 2e-2 tolerance).  PSUM accumulates fp32.
Output is written fp16 and upcast on host (adds Do there too).

Per core (4 batches): 16 conv matmuls + 16 output matmuls per batch =
128 matmuls-512 total (vs 452 in the previous version).  Output DMAs are
staged 4 output tiles per dma_start to amortize DGE cost; evacuation
copies rotate across DVE/Pool/Activation engines.
"""

import os
import sys

sys.path.insert(0, "/opt/trn_rl_repo")

import numpy as np

# problem dims (hardcoded per harness contract)
B, T, N, M = 32, 2048, 512, 512
NCORES = 8
BLOC = B // NCORES          # batches per core
LH = 64                     # direct head lags [0, LH); partition-quadrant aligned
R = 128 - LH                # tail rank (stacked into the same 128 contraction)
NTILE = 4                   # conv lag tiles of 128 -> tail window
WT = NTILE * 128            # tail lags [LH, LH+WT)
RPAD = LH + WT - 1          # 591: left zero pad of x
NV = T + 512                # diag buffer v-range [0, NV)
XROWS = NV + 127            # xpad rows so the diag load never reads OOB
NCH = T // 128              # output chunks per batch
NSC = T // 512              # superchunks per batch

MODE = os.environ.get("K_MODE", "f16")  # f16 | f8far
NLAG = NTILE  # kept for test.py cache-key compatibility
NV8 = 2176                  # fp8 far-tail diag buffer v-range (f8far mode)
XROWS8 = NV8 + 127


def build_program(mode=MODE):
    import concourse.tile as tile
    from concourse import bacc, mybir

    f16 = mybir.dt.float16
    f32 = mybir.dt.float32

    f8 = mybir.dt.float8e4
    f8far = mode == "f8far"
    nu16 = 2 if f8far else NTILE    # lag tiles kept fp16

    nc = bacc.Bacc("TRN2", target_bir_lowering=False, debug=False)
    xpad_t = nc.dram_tensor("xpad", [BLOC, XROWS], f16, kind="ExternalInput")
    hcomb_t = nc.dram_tensor("hcomb", [128, M], f16, kind="ExternalInput")
    ucomb_t = nc.dram_tensor("ucomb", [128, nu16 * R], f16, kind="ExternalInput")
    if f8far:
        xpad8_t = nc.dram_tensor("xpad8", [BLOC, XROWS8], f8, kind="ExternalInput")
        ucomb8_t = nc.dram_tensor("ucomb8", [128, 2 * R], f8, kind="ExternalInput")
    y_t = nc.dram_tensor("y", [BLOC, T, M], f16, kind="ExternalOutput")

    VCH = 512                   # v-granularity of xsh loads (1KB descriptors)
    with tile.TileContext(nc) as tc:
        with (
            tc.tile_pool(name="xsh", bufs=1) as xsh_pool,
            tc.tile_pool(name="w", bufs=1) as wpool,
            tc.tile_pool(name="psum", bufs=1, space="PSUM") as psum_pool,
            tc.tile_pool(name="stage", bufs=1) as stage_pool,
        ):
            # ---- load plan (critical-first, 3 queues round-robin) ----
            xsh = []
            xsh8 = []
            for b in range(BLOC):
                t_ = xsh_pool.tile([128, NV], f16, tag=f"xshb{b}", name=f"xsh{b}")
                xsh.append(t_)
                if f8far:
                    t8 = xsh_pool.tile(
                        [128, NV8], f8, tag=f"xsh8b{b}", name=f"xsh8{b}"
                    )
                    xsh8.append(t8)
            ucomb_sb = wpool.tile([128, nu16 * R], f16, tag="ucomb", name="ucomb_sb")
            if f8far:
                ucomb8_sb = wpool.tile([128, 2 * R], f8, tag="ucomb8", name="ucomb8_sb")
            hcomb_sb = wpool.tile([128, M], f16, tag="hcomb", name="hcomb_sb")

            engines = [nc.sync, nc.scalar, nc.gpsimd]
            ei = 0

            def dma(eng, out_ap, in_ap):
                eng.dma_start(out=out_ap, in_=in_ap)

            def load_xchunk(b, v0, eng):
                nvc = min(VCH, NV - v0)
                in_ap = xpad_t.ap().copy()
                from bass_rust import VecI64Pair
                in_ap.ap = VecI64Pair([[1, 128], [1, nvc]])
                in_ap.offset = b * XROWS + v0
                dma(eng, xsh[b][:, v0 : v0 + nvc], in_ap)

            def load_x8chunk(b, v0, eng):
                nvc = min(VCH, NV8 - v0)
                in_ap = xpad8_t.ap().copy()
                from bass_rust import VecI64Pair
                in_ap.ap = VecI64Pair([[1, 128], [1, nvc]])
                in_ap.offset = b * XROWS8 + v0
                dma(eng, xsh8[b][:, v0 : v0 + nvc], in_ap)

            # ucomb first (needed by the very first conv matmul); first x
            # chunks on sync+gpsimd (scalar's queue is blocked early by its
            # ACT_TABLE_LOAD).
            dma(nc.sync, ucomb_sb[:], ucomb_t.ap())
            dma(nc.scalar, hcomb_sb[:], hcomb_t.ap())
            nvch = (NV + VCH - 1) // VCH
            # b0 fully first (it feeds the first conv groups), alternating
            # the two fast queues per chunk so consecutive windows land in
            # parallel; then b1; b2/b3 spread over all three queues.
            le01 = [nc.sync, nc.gpsimd]
            le23 = [nc.sync, nc.gpsimd, nc.scalar]
            for b in (0, 1):
                for v in range(nvch):
                    load_xchunk(b, v * VCH, le01[ei % 2])
                    ei += 1
            ei = 0
            for v in range(nvch):
                for b in (2, 3):
                    load_xchunk(b, v * VCH, le23[ei % 3])
                    ei += 1

            # ---- compute ----
            # NOTE: GpSimd (Pool) cannot access PSUM, so evacuation copies
            # rotate across DVE + Activation only; y-write DMA issue goes to
            # SP + Pool to keep those two engines free for copies.
            evac_engines = [nc.vector, nc.scalar]
            ydma_engines = [nc.sync, nc.gpsimd]
            ci = 0
            yi = 0
            def zcopy(b, s, zt):
                nonlocal ci
                eng = evac_engines[ci % 2]
                ci += 1
                w0 = 512 + 512 * s
                if eng is nc.scalar:
                    eng.copy(xsh[b][LH:128, w0 : w0 + 512], zt[LH:128, :M])
                else:
                    eng.tensor_copy(xsh[b][LH:128, w0 : w0 + 512], zt[LH:128, :M])

            prevzt = {}

            def conv_group(b, s):
                # write z^T into PSUM partitions 64..127 directly
                # (tile_position col=64) so the copy to SBUF partitions
                # 64..127 never crosses partitions.  All PSUM goes through
                # one [128, 2M] (2-bank) tag so conv and out phases share
                # the 8 banks without overcommitting.
                zt = psum_pool.tile([128, M], f32, tag="zt", bufs=2, name="zt")
                for i, l in enumerate(range(NTILE - 1, -1, -1)):
                    v = 384 + 512 * s - 128 * l
                    nc.tensor.matmul(
                        zt[LH:128, :M],
                        lhsT=ucomb_sb[:, l * R : (l + 1) * R],
                        rhs=xsh[b][:, v : v + 512],
                        start=(i == 0),
                        stop=(i == NTILE - 1),
                    )
                # CRITICAL ORDER: zcopy(b, s) overwrites diag-buffer cells
                # (partitions 64..127, window [512+512s, 1024+512s)) that
                # conv(b, s+1) still reads as x data, so zcopy(b, s-1) is
                # emitted only now (after this group's matmuls), and
                # zcopy(b, NSC-1) right after its own group.
                if s > 0:
                    zcopy(b, s - 1, prevzt[b])
                prevzt[b] = zt
                if s == NSC - 1:
                    zcopy(b, s, zt)

            from bass_rust import VecI64Pair

            def out_pair(b, c0, osb, off, split=False):
                # two output-chunk matmuls into one 2-bank PSUM tile, then a
                # single fused (2-chunk-wide) evacuation copy -- halves the
                # per-op fixed cost on the two PSUM-capable engines.  With
                # split=True the two halves are copied by both engines in
                # parallel instead (shorter latency for the kernel tail).
                nonlocal ci
                pp = psum_pool.tile([128, 2 * M], f32, tag="po", bufs=3, name="pp")
                for k in (0, 1):
                    w = 512 + 128 * (c0 + k)
                    nc.tensor.matmul(
                        pp[:, k * M : (k + 1) * M],
                        lhsT=xsh[b][:, w : w + 128],
                        rhs=hcomb_sb[:],
                        start=True,
                        stop=True,
                    )
                if split:
                    nc.vector.tensor_copy(osb[:, off : off + M], pp[:, :M])
                    nc.scalar.copy(osb[:, off + M : off + 2 * M], pp[:, M:])
                else:
                    eng = evac_engines[ci % 2]
                    ci += 1
                    if eng is nc.scalar:
                        eng.copy(osb[:, off : off + 2 * M], pp[:])
                    else:
                        eng.tensor_copy(osb[:, off : off + 2 * M], pp[:])

            def out_group(b, g, ydma_eng=None, tail=False):
                # one staged y write of 4 output chunks (2 fused pairs)
                nonlocal yi
                tag, bufs = ("osbt", 2) if tail else ("osb", 6)
                osb = stage_pool.tile([128, 4 * M], f16, tag=tag, bufs=bufs, name=tag)
                out_pair(b, 4 * g, osb, 0, split=tail)
                out_pair(b, 4 * g + 2, osb, 2 * M, split=tail)
                dst = y_t.ap().copy()
                dst.ap = VecI64Pair([[M, 128], [128 * M, 4], [1, M]])
                dst.offset = b * T * M + g * 512 * M
                eng = ydma_eng or ydma_engines[yi % 2]
                yi += 1
                eng.dma_start(out=dst, in_=osb[:])

            # Schedule: conv(pair1) -> [out(pair1) interleaved with
            # conv(pair2)] (that section is PE-bound: evac demand of the out
            # groups fits beside the conv groups' zcopies) -> out(pair2)
            # with a short-latency tail.
            for b in (0, 1):
                for s in range(NSC):
                    conv_group(b, s)
            og = [(b, g) for b in (0, 1) for g in range(NCH // 4)]
            cg = [(b, s) for b in (2, 3) for s in range(NSC)]
            for i in range(len(og)):
                out_group(*og[i])
                conv_group(*cg[i])
            for g in range(NCH // 4):
                out_group(2, g, tail=(g == 3))
                out_group(3, g,
                          ydma_eng=nc.scalar if g == 3 else None,
                          tail=(g == 3))

    nc.compile()
    return nc


def host_weights(lnl_re, lnl_im, W_r, W_i, C, D, Do, mode=MODE):
    """Impulse response head + SVD-factored tail, float64 math."""
    lnl = lnl_re.astype(np.float64) + 1j * lnl_im.astype(np.float64)
    W = W_r.astype(np.float64) + 1j * W_i.astype(np.float64)
    Winv = np.linalg.inv(W)
    A_re = np.ascontiguousarray(Winv.real.T) @ C.astype(np.float64)
    A_im = np.ascontiguousarray(Winv.imag.T) @ C.astype(np.float64)
    j = np.arange(LH + WT, dtype=np.float64)
    P = np.exp(np.outer(j, lnl))
    H = P.real @ A_re - P.imag @ A_im                 # (LH+WT, M)
    H[0] += D[0].astype(np.float64)

    Hh = H[:LH]
    U, S, Vt = np.linalg.svd(H[LH:], full_matrices=False)
    sq = np.sqrt(S[:R])
    Uf = U[:, :R] * sq                                # (WT, R)
    Vf = sq[:, None] * Vt[:R]                         # (R, M)

    hcomb = np.concatenate([Hh[::-1], Vf], axis=0).astype(np.float16)
    ucomb = (
        Uf.reshape(NTILE, 128, R)[:, ::-1, :]
        .transpose(1, 0, 2)
        .reshape(128, NTILE * R)
        .astype(np.float16)
    )
    return {
        "hcomb": np.ascontiguousarray(hcomb),
        "ucomb": np.ascontiguousarray(ucomb),
    }


def make_in_maps(x, weights):
    x16 = x[:, :, 0].astype(np.float16)               # (B, T)
    in_maps = []
    for c in range(NCORES):
        xpad = np.zeros((BLOC, XROWS), np.float16)
        xpad[:, RPAD : RPAD + T] = x16[c * BLOC : (c + 1) * BLOC]
        im = dict(weights)
        im["xpad"] = xpad
        in_maps.append(im)
    return in_maps


_prog_cache = {}


def kernel(x, lnl_re, lnl_im, W_r, W_i, C, D, Do):
    from concourse.bass_utils import run_bass_kernel_spmd

    x = np.asarray(x)
    lnl_re, lnl_im = np.asarray(lnl_re), np.asarray(lnl_im)
    W_r, W_i = np.asarray(W_r), np.asarray(W_i)
    C, D, Do = np.asarray(C), np.asarray(D), np.asarray(Do)

    key = (NLAG, MODE)
    if key not in _prog_cache:
        _prog_cache[key] = build_program()
    nc = _prog_cache[key]

    weights = host_weights(lnl_re, lnl_im, W_r, W_i, C, D, Do)
    in_maps = make_in_maps(x, weights)
    res = run_bass_kernel_spmd(nc, in_maps, core_ids=list(range(NCORES)))
    y = np.concatenate([res.results[i]["y"] for i in range(NCORES)], axis=0)
    y = y.astype(np.float32) + Do.astype(np.float32)[None, None, :]
    return np.ascontiguousarray(y)
